# revision 1
# baseline (speedup 1.0000x reference)
"""Trainium2 Bass kernel for LIFNet (leaky-integrator net, no spiking).

Math: the module is linear, and the leaky integration L (a causal LTI filter
along T) commutes with the per-timestep linear layers:

    V2 = L(L(batch @ W1^T) @ W2^T) = (L^2)(batch @ (W2 @ W1)^T)

with Wc = W2 @ W1 of shape [10, 784].  L^2 has impulse response
h[m] = beta^2 (m-1) alpha^(m-2) (m >= 2), which decays below f32 noise by
lag ~128, so the filter is applied as a banded blocked matmul with two
constant 128x128 blocks (intra-block R0, previous-block R1).

Sharding (balanced, max-core bytes minimized): each core gets 12 full b's
(cores 0-7 -> b 12c..12c+11, covering b 0..95) plus HALF (by T) of one of
the remaining b's 96..99: core c processes b 96+c//2, T-half c%2, as a
1152-t segment (128 warm-up t's for the upper half; the filter impulse
response is < 1e-16 beyond lag ~228, so starting the recursion 128 t's
early is exact to f32).

Device work per core:
  - x is bf16 (host-converted) to halve HBM traffic; two half-b SWDGE
    DMAs per b with per-partition-contiguous 14 KB runs (descriptor-
    efficient, keeps up to 8 transfers queued, and lets compute on the
    first half overlap the DMA of the second).
  - z^T = Wc @ x^T via PE matmuls, 7 d-chunks (K=112) issued into the 4
    column-groups of the PE array (tile_position=(0, 32q)) so up to 4
    chunk-matmuls stream concurrently; per-quadrant partials [106, N]
    land in one PSUM tile and a tiny selector matmul (S[106, 10], bf16)
    sums the quadrants into z^T [10, N].
  - b's are processed in groups of 4, stacked at 32-partition offsets in
    the z^T staging tile [128, 2048], so the PE transpose ([128, 128],
    bf16) and the banded filter matmuls (M=128, bf16) amortize over 4
    b's.  The T-segment forms a final narrow (PW=32, 9-block) group so
    the end-of-stream critical path is minimal; each group's stage-2/3
    is emitted after the 2nd b of the NEXT group (the PE stream is
    in-order, so emission order controls head-of-line blocking).
  - All constants load over the same SWDGE FIFO BEFORE the bulk input
    stream (HWDGE queues are starved while the SWDGE queue is nonempty
    on trn2); V2^T slices DMA out on the scalar HWDGE queue.
  - Host re-assembles [100, 2000, 10].
"""

import sys

import numpy as np

for _p in ("/opt/trn_rl_repo",):
    if _p not in sys.path:
        sys.path.append(_p)

B, T, DIN, H1, H2 = 100, 2000, 784, 100, 10
ALPHA, BETA = 0.7, 0.3

NCORES = 8
BPF = 12            # full b's per core (8 * 12 = 96)
BGRP = 4            # b's stacked per stage-2/3 group (quadrant offsets)
DC = 112            # d-chunk width (784 = 7 * 112), partition dim of x tiles
NDC = DIN // DC     # 7
QS = 106            # stacked partials: quadrant q rows 32q .. 32q+9
TG = 500            # t-columns per z-matmul group (N <= 512)
NTG = T // TG       # 4
TB = 128            # t'-block for the filter stage
NTB = (T + TB - 1) // TB  # 16
TPADF = NTB * TB    # 2048 free-dim padding for the z^T staging buffer
TS = 1152           # segment length (9 t-blocks): 1024 lower / 128 warm-up
NTBS = TS // TB     # 9
SGU = 288           # segment z-matmul unit width (4 * 288 = 1152)
NSG = TS // SGU     # 4
SEG_LO = 1024       # lower-half cores emit t < 1024
SEG_W0 = TB         # upper-half warm-up t's (discarded)

_CACHE: dict = {}


def _filter_blocks() -> np.ndarray:
    """R = [R1 | R0] as [128, 256] f32: rhs blocks for the filter matmuls.

    out[o, t'] += sum_tl z_block[tl, o] * R[tl, t'] with R[tl, t'] =
    h[lag], lag = (t' - tl) + 128 for R1 (z from previous t-block) and
    (t' - tl) for R0 (intra-block, strictly causal).
    """
    m = np.arange(512, dtype=np.float64)
    h = np.zeros(512)
    h[2:] = BETA * BETA * (m[2:] - 1.0) * ALPHA ** (m[2:] - 2.0)
    tl = np.arange(TB)[:, None]
    tp = np.arange(TB)[None, :]
    r1 = h[tp - tl + TB]
    lag0 = tp - tl
    r0 = np.where(lag0 >= 2, h[np.clip(lag0, 0, None)], 0.0)
    return np.concatenate([r1, r0], axis=1).astype(np.float32)


def _build(reps: int = 1):
    """Build + compile the per-core Bass kernel (shared by all 8 cores)."""
    from contextlib import ExitStack

    import concourse.tile as tile
    from concourse import bacc, mybir

    f32 = mybir.dt.float32
    bf16 = mybir.dt.bfloat16
    nc = bacc.Bacc(
        "TRN2", target_bir_lowering=False, debug=False, num_devices=NCORES
    )

    xT = nc.dram_tensor(
        "xT", [BPF, 2, DC, NDC, T // 2], bf16, kind="ExternalInput"
    )
    xS = nc.dram_tensor(
        "xS", [2, DC, NDC, TS // 2], bf16, kind="ExternalInput"
    )
    wct = nc.dram_tensor("wct", [DC, NDC * H2], bf16, kind="ExternalInput")
    rh = nc.dram_tensor("rh", [TB, 2 * TB], bf16, kind="ExternalInput")
    eye = nc.dram_tensor("eye", [TB, TB], bf16, kind="ExternalInput")
    sel = nc.dram_tensor("sel", [QS, H2], bf16, kind="ExternalInput")
    vout = nc.dram_tensor(
        "vout", [(BPF + 1) * H2, T], bf16, kind="ExternalOutput"
    )

    with tile.TileContext(nc) as tc, ExitStack() as ctx:
        const = ctx.enter_context(tc.tile_pool(name="const", bufs=1))
        xpool = ctx.enter_context(tc.tile_pool(name="xp", bufs=9))
        xspool = ctx.enter_context(tc.tile_pool(name="xs", bufs=2))
        zsump = ctx.enter_context(tc.tile_pool(name="zsum", bufs=2))
        ring = ctx.enter_context(tc.tile_pool(name="ring", bufs=1))
        zbp = ctx.enter_context(tc.tile_pool(name="zbp", bufs=2))
        vsb = ctx.enter_context(tc.tile_pool(name="vsb", bufs=3))
        zps4 = ctx.enter_context(tc.tile_pool(name="zps4", bufs=2, space="PSUM"))
        zredp = ctx.enter_context(tc.tile_pool(name="zred", bufs=2, space="PSUM"))
        tpsum = ctx.enter_context(tc.tile_pool(name="tps", bufs=2, space="PSUM"))
        vpsum = ctx.enter_context(tc.tile_pool(name="vps", bufs=2, space="PSUM"))

        # consts on the SAME SWDGE FIFO as the bulk input, issued first
        wct_sb = const.tile([DC, NDC * H2], bf16, tag="wct")
        nc.gpsimd.dma_start(wct_sb[:], wct.ap())
        rh_sb = const.tile([TB, 2 * TB], bf16, tag="rh")
        nc.gpsimd.dma_start(rh_sb[:], rh.ap())
        eye_sb = const.tile([TB, TB], bf16, tag="eye")
        nc.gpsimd.dma_start(eye_sb[:], eye.ap())
        sel_sb = const.tile([QS, H2], bf16, tag="sel")
        nc.gpsimd.dma_start(sel_sb[:], sel.ap())

        # Two-deep manual ring: quadrant rows 32q+10..31 and t-pad cols
        # of the z^T staging tile must stay zero, memset once.
        zts_ring = []
        for i in range(2):
            zt = ring.tile([TB, TPADF], bf16, tag=f"zts{i}", name=f"zts{i}")
            nc.vector.memset(zt[:], 0.0)
            zts_ring.append(zt)

        HF = NDC * (T // 2)  # free-size of one half-b in the xt tile

        def zunits(q, zts, xparts, widths, cstride, zsum=None, toff0=0):
            """Stage-1 z-matmul units: each unit streams `w` t-columns of
            all 7 d-chunks into the 4 PE column groups, then reduces the
            quadrant partials with the selector matmul into zts rows
            32q..32q+9."""
            if zsum is None:
                zsum = zsump.tile([QS, sum(widths)], bf16, tag=f"zs{cstride}")
            toff = toff0
            for i, w in enumerate(widths):
                u = toff0 // max(widths) + i
                zp4 = zps4.tile([QS, TG], f32, tag="zp4")
                xt, xoff = xparts[i]
                for c in range(NDC):
                    cq = c % 4
                    nc.tensor.matmul(
                        zp4[32 * cq : 32 * cq + H2, 0:w],
                        wct_sb[:, c * H2 : (c + 1) * H2],
                        xt[:, xoff + c * cstride : xoff + c * cstride + w],
                        start=(c < 4),
                        stop=(c >= 3),
                        tile_position=(0, 32 * cq),
                    )
                if u % 2 == 0:
                    nc.scalar.copy(
                        zsum[:, toff : toff + w], zp4[:, 0:w]
                    )
                else:
                    nc.vector.tensor_copy(
                        zsum[:, toff : toff + w], zp4[:, 0:w]
                    )
                zr = zredp.tile([H2, TG], f32, tag="zr")
                nc.tensor.matmul(
                    zr[:, 0:w],
                    sel_sb[:],
                    zsum[:, toff : toff + w],
                    start=True,
                    stop=True,
                )
                if u % 2 == 0:
                    nc.vector.tensor_copy(
                        zts[32 * q : 32 * q + H2, toff : toff + w],
                        zr[:, 0:w],
                    )
                else:
                    nc.scalar.copy(
                        zts[32 * q : 32 * q + H2, toff : toff + w],
                        zr[:, 0:w],
                    )
                toff += w

        def stage1(b, q, zts):
            # SWDGE (gpsimd) path; per-half tiles keep up to 8 transfers
            # outstanding on the queue and let compute on the first half
            # overlap the DMA of the second.
            xth = []
            for h in range(2):
                xt = xpool.tile([DC, HF], bf16, tag="xt")
                nc.gpsimd.dma_start(
                    xt[:].rearrange("p (c t) -> p c t", c=NDC),
                    xT.ap()[b, h],
                )
                xth.append(xt)
            xparts = [
                (xth[g // 2], (g % 2) * TG) for g in range(NTG)
            ]
            zunits(q, zts, xparts, [TG] * NTG, T // 2)

        def seg_pipeline(zts, flush_outs):
            """The final T-segment, fully pipelined: half-0 units,
            transposes and filter blocks run while half-1 streams, so
            only ~2 z-units + 5 t-blocks of stage-2/3 remain after the
            last input byte lands."""
            PW = 32
            xh = []
            for h in range(2):
                xt = xspool.tile([DC, NDC * (TS // 2)], bf16, tag="xts")
                nc.gpsimd.dma_start(
                    xt[:].rearrange("p (c t) -> p c t", c=NDC), xS.ap()[h]
                )
                xh.append(xt)
            flush_outs()
            zsum = zsump.tile([QS, TS], bf16, tag="zss")
            zb = zbp.tile([TB, NTBS * PW], bf16, tag=f"zb{PW}")
            v2 = vsb.tile([PW, TS], bf16, tag=f"v2{PW}")

            def units(us):
                xparts = [(xh[u // 2], (u % 2) * SGU) for u in us]
                zunits(
                    0, zts, xparts, [SGU] * len(us), TS // 2,
                    zsum=zsum, toff0=us[0] * SGU,
                )

            def trans(jlo, jhi):
                for j in range(jlo, jhi):
                    ztp = tpsum.tile([TB, PW], bf16, tag="ztp")
                    nc.tensor.transpose(
                        ztp[:],
                        zts[0:PW, j * TB : (j + 1) * TB],
                        eye_sb[0:PW, 0:PW],
                    )
                    if j % 2 == 0:
                        nc.scalar.copy(zb[:, j * PW : (j + 1) * PW], ztp[:])
                    else:
                        nc.vector.tensor_copy(
                            zb[:, j * PW : (j + 1) * PW], ztp[:]
                        )

            def filt(jlo, jhi):
                for j in range(jlo, jhi):
                    vp = vpsum.tile([PW, TB], f32, tag="vp")
                    n_mm = 2 if j > 0 else 1
                    mm = 0
                    for roff, jj in ((0, j - 1), (TB, j)):
                        if jj < 0:
                            continue
                        nc.tensor.matmul(
                            vp[:],
                            zb[:, jj * PW : (jj + 1) * PW],
                            rh_sb[:, roff : roff + TB],
                            start=(mm == 0),
                            stop=(mm == n_mm - 1),
                        )
                        mm += 1
                    w = min(TB, TS - j * TB)
                    if j % 2 == 0:
                        nc.scalar.copy(
                            v2[:, j * TB : j * TB + w], vp[:, 0:w]
                        )
                    else:
                        nc.vector.tensor_copy(
                            v2[:, j * TB : j * TB + w], vp[:, 0:w]
                        )

            units([0, 1])   # half 0: z cols 0..576 -> t-blocks 0..3
            trans(0, 4)
            filt(0, 4)
            units([2, 3])   # half 1 (after its DMA): cols 576..1152
            trans(4, NTBS)
            filt(4, NTBS)
            nc.scalar.dma_start(
                vout.ap()[BPF * H2 : (BPF + 1) * H2, 0:TS], v2[0:H2, :]
            )

        def stage23(bs, zts, ntb, tw, defer_out=True):
            # z[t, p] (p = 32q+o) per 128-t-block via PE transpose of the
            # group's stacked z^T rows.  PW: partition width of the
            # stacked stage (narrow for the final segment group).
            PW = 32 * len(bs)
            zb = zbp.tile([TB, ntb * PW], bf16, tag=f"zb{PW}")
            for j in range(ntb):
                ztp = tpsum.tile([TB, PW], bf16, tag="ztp")
                nc.tensor.transpose(
                    ztp[:],
                    zts[0:PW, j * TB : (j + 1) * TB],
                    eye_sb[0:PW, 0:PW],
                )
                if j % 2 == 0:
                    nc.scalar.copy(zb[:, j * PW : (j + 1) * PW], ztp[:])
                else:
                    nc.vector.tensor_copy(
                        zb[:, j * PW : (j + 1) * PW], ztp[:]
                    )

            # V2^T[p, 128-t'-block] = sum over prev/current z t-blocks,
            # whole b-group at once (M = PW output rows per matmul).
            v2 = vsb.tile([PW, tw], bf16, tag=f"v2{PW}")
            for j in range(ntb):
                vp = vpsum.tile([PW, TB], f32, tag="vp")
                n_mm = 2 if j > 0 else 1
                mm = 0
                for roff, jj in ((0, j - 1), (TB, j)):
                    if jj < 0:
                        continue
                    nc.tensor.matmul(
                        vp[:],
                        zb[:, jj * PW : (jj + 1) * PW],
                        rh_sb[:, roff : roff + TB],
                        start=(mm == 0),
                        stop=(mm == n_mm - 1),
                    )
                    mm += 1
                w = min(TB, tw - j * TB)
                if j % 2 == 0:
                    nc.scalar.copy(v2[:, j * TB : j * TB + w], vp[:, 0:w])
                else:
                    nc.vector.tensor_copy(
                        v2[:, j * TB : j * TB + w], vp[:, 0:w]
                    )

            if defer_out:
                outq.append((v2, bs, tw))
            else:
                for q, b in enumerate(bs):
                    nc.scalar.dma_start(
                        vout.ap()[b * H2 : (b + 1) * H2, 0:tw],
                        v2[32 * q : 32 * q + H2, :],
                    )

        # The T-segment group is LAST (its narrow 9-block stage-2/3 is
        # the cheapest possible end-of-stream chain); each group's
        # stage-2/3 is emitted after the 2nd b of the NEXT group, and
        # the penultimate group's is flushed BEFORE the segment's
        # stage-1 so the in-order PE stream has no ready work queued
        # behind the last DMA wait.
        groups = [
            list(range(k * BGRP, (k + 1) * BGRP)) for k in range(BPF // BGRP)
        ] + ["SEG"]
        for rep in range(reps):
            pending = None
            outq = []
            for gi, bs in enumerate(groups):
                zts = zts_ring[gi % 2]
                if bs == "SEG":
                    if pending is not None:
                        stage23(*pending)
                        pending = None

                    def flush_outs():
                        # all bulk input DMAs are issued; release the
                        # buffered outputs (HBM writes no longer
                        # interleave with the input read stream)
                        for v2d, obs, otw in outq:
                            for q, b in enumerate(obs):
                                nc.scalar.dma_start(
                                    vout.ap()[b * H2 : (b + 1) * H2, 0:otw],
                                    v2d[32 * q : 32 * q + H2, :],
                                )
                        outq.clear()

                    seg_pipeline(zts, flush_outs)
                    continue
                for q, b in enumerate(bs):
                    stage1(b, q, zts)
                    if q == 1 and pending is not None:
                        stage23(*pending)
                        pending = None
                if pending is not None:
                    stage23(*pending)
                    pending = None
                pending = (bs, zts, NTB, T)
            if pending is not None:
                stage23(*pending)

    nc.compile()
    return nc


def _prep_inputs(batch: np.ndarray, W1: np.ndarray, W2: np.ndarray):
    import ml_dtypes

    bf16 = ml_dtypes.bfloat16
    wc = (W2.astype(np.float64) @ W1.astype(np.float64)).astype(np.float32)
    # [112, 7*10]: wct[p, c*10+o] = Wc[o, 112c + p]
    wct = np.ascontiguousarray(
        wc.T.reshape(NDC, DC, H2).transpose(1, 0, 2).reshape(DC, NDC * H2)
    ).astype(bf16)
    rh = _filter_blocks().astype(bf16)
    eye = np.eye(TB, dtype=np.float32).astype(bf16)
    sel = np.zeros((QS, H2), np.float32)
    for q in range(4):
        for i in range(H2):
            sel[32 * q + i, i] = 1.0
    sel = sel.astype(bf16)

    # full b's 0..95: [8, 12, 2, 112, 7, 1000]: core, b, t-half,
    # d%112 (partitions), d-chunk, t-within-half
    xt = np.ascontiguousarray(
        batch[: NCORES * BPF]
        .reshape(NCORES, BPF, 2, T // 2, NDC, DC)
        .transpose(0, 1, 2, 5, 4, 3)
    ).astype(bf16)

    # T-segments of b's 96..99: core c gets b 96+c//2, half c%2.
    # Lower half: t 0..1152 (host keeps t<1024).  Upper half: t
    # 896..2048 (first 128 are filter warm-up; host keeps t>=1024).
    seg = np.zeros((NCORES, TS, DIN), np.float32)
    for c in range(NCORES):
        be = NCORES * BPF + c // 2
        if c % 2 == 0:
            seg[c] = batch[be, 0:TS]
        else:
            seg[c, 0 : T - (SEG_LO - SEG_W0)] = batch[be, SEG_LO - SEG_W0 :]
    xs = np.ascontiguousarray(
        seg.reshape(NCORES, 2, TS // 2, NDC, DC).transpose(0, 1, 4, 3, 2)
    ).astype(bf16)
    return xt, xs, wct, rh, eye, sel


def kernel(batch: np.ndarray, W1: np.ndarray, W2: np.ndarray) -> np.ndarray:
    from concourse import bass_utils

    if "nc" not in _CACHE:
        _CACHE["nc"] = _build()
    nc = _CACHE["nc"]

    xt, xs, wct, rh, eye, sel = _prep_inputs(batch, W1, W2)
    in_maps = [
        {
            "xT": xt[i],
            "xS": xs[i],
            "wct": wct,
            "rh": rh,
            "eye": eye,
            "sel": sel,
        }
        for i in range(NCORES)
    ]
    res = bass_utils.run_bass_kernel_spmd(
        nc, in_maps, core_ids=list(range(NCORES)), **_CACHE.get("run_kwargs", {})
    )
    _CACHE["last_result"] = res

    out = np.empty((B, T, H2), np.float32)
    for c in range(NCORES):
        vo = res.results[c]["vout"].astype(np.float32)  # [130, 2000]
        out[c * BPF : (c + 1) * BPF] = (
            vo[: BPF * H2].reshape(BPF, H2, T).transpose(0, 2, 1)
        )
        segv = vo[BPF * H2 :]  # [10, 2000]; valid cols 0..TS
        be = NCORES * BPF + c // 2
        if c % 2 == 0:
            out[be, 0:SEG_LO] = segv[:, 0:SEG_LO].T
        else:
            out[be, SEG_LO:T] = segv[:, SEG_W0 : SEG_W0 + (T - SEG_LO)].T
    return out



# revision 15
# speedup vs baseline: 1.6281x; 1.6281x over previous
"""Trainium2 Bass kernel for LIFNet (leaky-integrator net, no spiking).

Math: the module is linear, and the leaky integration L (a causal LTI filter
along T) commutes with the per-timestep linear layers:

    V2 = L(L(batch @ W1^T) @ W2^T) = (L^2)(batch @ (W2 @ W1)^T)

with Wc = W2 @ W1 of shape [10, 784].  L^2 has impulse response
h[m] = beta^2 (m-1) alpha^(m-2) (m >= 2), which decays below f32 noise by
lag ~128, so the filter is applied as a banded blocked matmul with two
constant 128x128 blocks (intra-block R0, previous-block R1).

Sharding (balanced, max-core bytes minimized): each core gets 12 full b's
(cores 0-7 -> b 12c..12c+11, covering b 0..95) plus HALF (by T) of one of
the remaining b's 96..99: core c processes b 96+c//2, T-half c%2, as a
1152-t segment (128 warm-up t's for the upper half; the filter impulse
response is < 1e-16 beyond lag ~228, so starting the recursion 128 t's
early is exact to f32).

Device work per core (the stream is HBM-read bound, so x is fp8-e3m4,
host-encoded at 2x scale -- measured end-to-end rel err ~1.4e-2 vs the
2e-2 gate; weights stay bf16, the PE supports mixed bf16xfp8 exactly):
  - one SWDGE DMA per b ([112 part, 14 KB contiguous lines]); the first
    b's DMA is issued BEFORE the two packed const DMAs so the const
    descriptor generation overlaps the first bulk transfer.
  - z^T = Wc @ x^T via PE matmuls: per 500-t unit, all 7 d-chunks
    (K=112) accumulate into ONE PSUM quadrant (rows 32q..32q+9 of a
    [106, 500] bank tile, tile_position=(0, 32q), q = unit%4 so up to 4
    units' chains interleave on the array); the PSUM band is copied
    (f32->fp16) straight into the z^T staging tile -- no selector
    matmul, no intermediate stacking copies.
  - b's are processed in bands of 4, packed at 10-partition offsets
    (rows 10*(b%4)..+10) in the staging tile [40, 2048], so the PE
    transpose ([40,128]->[128,40]) and the banded filter matmuls
    (M=40) amortize over 4 b's and the band's output leaves as a
    single [40, 2000] fp16 DMA (deferred until all input DMAs are
    queued).  The T-segment forms a final narrow (PW=10, 9-block)
    group so the end-of-stream critical path is minimal; each band's
    stage-2/3 is emitted after the 2nd b of the NEXT band (the PE
    stream is in-order, so emission order controls head-of-line
    blocking).
  - All constants load over the same SWDGE FIFO as the bulk input
    (HWDGE queues are starved while the SWDGE queue is nonempty on
    trn2); V2^T band slices DMA out on the scalar HWDGE queue.
  - Host re-assembles [100, 2000, 10].
"""

import sys

import numpy as np

for _p in ("/opt/trn_rl_repo",):
    if _p not in sys.path:
        sys.path.append(_p)

B, T, DIN, H1, H2 = 100, 2000, 784, 100, 10
ALPHA, BETA = 0.7, 0.3

NCORES = 8
BPF = 12            # full b's per core (8 * 12 = 96)
BGRP = 4            # b's per stage-2/3 band (10-partition offsets)
PW = BGRP * H2      # 40: partition width of band stage-2/3
DC = 112            # d-chunk width (784 = 7 * 112), partition dim of x tiles
NDC = DIN // DC     # 7
XS = 2.0            # host pre-scale of x before fp8-e3m4 encode
TG = 500            # t-columns per z-matmul unit (PSUM bank max 512 f32)
NTG = T // TG       # 4
TB = 128            # t'-block for the filter stage
NTB = (T + TB - 1) // TB  # 16
TPADF = NTB * TB    # 2048 free-dim padding for the z^T staging buffer
TS = 1152           # segment length (9 t-blocks): 1024 lower / 128 warm-up
NTBS = TS // TB     # 9
SGU = 288           # segment z-matmul unit width (4 * 288 = 1152)
NSG = TS // SGU     # 4
SEG_LO = 1024       # lower-half cores emit t < 1024
SEG_W0 = TB         # upper-half warm-up t's (discarded)
RHF = 2 * TB        # rh cols in the packed const
CF = RHF + TB       # packed const free size (rh | eye-128)

_CACHE: dict = {}


def _filter_blocks() -> np.ndarray:
    """R = [R1 | R0] as [128, 256] f32: rhs blocks for the filter matmuls.

    out[o, t'] += sum_tl z_block[tl, o] * R[tl, t'] with R[tl, t'] =
    h[lag], lag = (t' - tl) + 128 for R1 (z from previous t-block) and
    (t' - tl) for R0 (intra-block, strictly causal).
    """
    m = np.arange(512, dtype=np.float64)
    h = np.zeros(512)
    h[2:] = BETA * BETA * (m[2:] - 1.0) * ALPHA ** (m[2:] - 2.0)
    tl = np.arange(TB)[:, None]
    tp = np.arange(TB)[None, :]
    r1 = h[tp - tl + TB]
    lag0 = tp - tl
    r0 = np.where(lag0 >= 2, h[np.clip(lag0, 0, None)], 0.0)
    return np.concatenate([r1, r0], axis=1).astype(np.float32)


def _build():
    """Build + compile the per-core Bass kernel (shared by all 8 cores)."""
    from contextlib import ExitStack

    import concourse.tile as tile
    from concourse import bacc, mybir

    f32 = mybir.dt.float32
    bf16 = mybir.dt.bfloat16
    fp16 = mybir.dt.float16
    fp8 = mybir.dt.float8e3
    nc = bacc.Bacc(
        "TRN2", target_bir_lowering=False, debug=False, num_devices=NCORES
    )

    xT = nc.dram_tensor("xT", [BPF, DC, NDC, T], fp8, kind="ExternalInput")
    xS = nc.dram_tensor("xS", [2, DC, NDC, TS // 2], fp8, kind="ExternalInput")
    wct = nc.dram_tensor("wct", [DC, NDC * H2], bf16, kind="ExternalInput")
    rheye = nc.dram_tensor("rheye", [TB, CF], fp16, kind="ExternalInput")
    vout = nc.dram_tensor(
        "vout", [(BPF + 1) * H2, T], fp16, kind="ExternalOutput"
    )

    with tile.TileContext(nc) as tc, ExitStack() as ctx:
        const = ctx.enter_context(tc.tile_pool(name="const", bufs=1))
        xpool = ctx.enter_context(tc.tile_pool(name="xp", bufs=6))
        xspool = ctx.enter_context(tc.tile_pool(name="xs", bufs=2))
        ring = ctx.enter_context(tc.tile_pool(name="ring", bufs=1))
        zbp = ctx.enter_context(tc.tile_pool(name="zbp", bufs=2))
        vsb = ctx.enter_context(tc.tile_pool(name="vsb", bufs=3))
        zps = ctx.enter_context(tc.tile_pool(name="zps", bufs=2, space="PSUM"))
        tpsum = ctx.enter_context(tc.tile_pool(name="tps", bufs=2, space="PSUM"))
        vpsum = ctx.enter_context(tc.tile_pool(name="vps", bufs=2, space="PSUM"))

        # First bulk DMA goes out BEFORE the consts: its data transfer
        # overlaps the consts' descriptor generation on the Q7.
        xt0 = xpool.tile([DC, NDC * T], fp8, tag="xt")
        nc.gpsimd.dma_start(
            xt0[:].rearrange("p (c t) -> p c t", c=NDC), xT.ap()[0]
        )

        # consts on the SAME SWDGE FIFO as the bulk input
        wct_sb = const.tile([DC, NDC * H2], bf16, tag="wct")
        nc.gpsimd.dma_start(wct_sb[:], wct.ap())
        rheye_sb = const.tile([TB, CF], fp16, tag="rheye")
        nc.gpsimd.dma_start(rheye_sb[:], rheye.ap())

        # Two-deep manual ring of z^T staging tiles.  Bands live at
        # 32-partition offsets (compute-engine partition bases must be
        # 32-aligned); rows 32q+10..31 and the t-pad cols must stay
        # zero (the full-width transpose contracts over all 128 rows),
        # memset once.
        zts_ring = []
        for i in range(2):
            zt = ring.tile([TB, TPADF], fp16, tag=f"zts{i}", name=f"zts{i}")
            nc.vector.memset(zt[:], 0.0)
            zts_ring.append(zt)

        eng = [0]

        def alt_copy(dst, src):
            # alternate PSUM->SBUF copies between the two copy engines
            if eng[0] % 2 == 0:
                nc.scalar.copy(dst, src)
            else:
                nc.vector.tensor_copy(dst, src)
            eng[0] += 1

        def zchains(zts, row0, parts, pos0=0):
            """Interleaved z-matmul unit chains: parts = per-unit
            (xt, xoff, w, cstride, toff).  The 7 d-chunks of every unit
            accumulate into PSUM band rows 32q..32q+9 (q = pos0+unit,
            tile_position=(0, 32q)); chunk MMs are emitted c-outer so
            LDWEIGHTS at one array position overlaps streaming at
            another.  Bands are then copied (f32->fp16) straight into
            the z^T staging tile -- no selector matmul."""
            zp = zps.tile([3 * 32 + H2, TG], f32, tag="zp")
            for c in range(NDC):
                for u, (xt, xoff, w, cstride, _) in enumerate(parts):
                    q = pos0 + u
                    nc.tensor.matmul(
                        zp[32 * q : 32 * q + H2, 0:w],
                        wct_sb[:, c * H2 : (c + 1) * H2],
                        xt[:, xoff + c * cstride : xoff + c * cstride + w],
                        start=(c == 0),
                        stop=(c == NDC - 1),
                        tile_position=(0, 32 * q),
                    )
            for u, (_, _, w, _, toff) in enumerate(parts):
                q = pos0 + u
                alt_copy(
                    zts[row0 : row0 + H2, toff : toff + w],
                    zp[32 * q : 32 * q + H2, 0:w],
                )



        def stage1(b, bq, zts):
            xt = xt0 if b == 0 else xpool.tile([DC, NDC * T], fp8, tag="xt")
            if b != 0:
                nc.gpsimd.dma_start(
                    xt[:].rearrange("p (c t) -> p c t", c=NDC), xT.ap()[b]
                )
            zchains(
                zts, 32 * bq,
                [(xt, u * TG, TG, T, u * TG) for u in range(NTG)],
            )

        def stage23(bs0, zts, ntb, tw, pw):
            """Transpose the staging tile per 128-t-block and apply the
            banded filter, whole band (pw partitions) at once."""
            zb = zbp.tile([TB, NTB * PW], fp16, tag="zb")
            for j in range(ntb):
                ztp = tpsum.tile([TB, TB], fp16, tag="ztp")
                if pw == PW:
                    # full-width transpose (bands live at 32-offsets;
                    # spare rows are zero), then compact 4x10 of the
                    # 128 columns into zb's dense 40 via a strided AP.
                    nc.tensor.transpose(
                        ztp[:],
                        zts[0:TB, j * TB : (j + 1) * TB],
                        rheye_sb[0:TB, RHF : RHF + TB],
                    )
                    alt_copy(
                        zb[:, j * PW : (j + 1) * PW].rearrange(
                            "p (g c) -> p g c", g=BGRP
                        ),
                        ztp[:].rearrange("p (g c) -> p g c", g=BGRP)[
                            :, :, 0:H2
                        ],
                    )
                else:
                    nc.tensor.transpose(
                        ztp[:, 0:pw],
                        zts[0:pw, j * TB : (j + 1) * TB],
                        rheye_sb[0:pw, RHF : RHF + pw],
                    )
                    alt_copy(zb[:, j * PW : j * PW + pw], ztp[:, 0:pw])

            v2 = vsb.tile([PW, TPADF], fp16, tag="v2")
            for j in range(ntb):
                vp = vpsum.tile([PW, TB], f32, tag="vp")
                n_mm = 2 if j > 0 else 1
                mm = 0
                for roff, jj in ((0, j - 1), (TB, j)):
                    if jj < 0:
                        continue
                    nc.tensor.matmul(
                        vp[0:pw, :],
                        zb[:, jj * PW : jj * PW + pw],
                        rheye_sb[:, roff : roff + TB],
                        start=(mm == 0),
                        stop=(mm == n_mm - 1),
                    )
                    mm += 1
                w = min(TB, tw - j * TB)
                alt_copy(v2[0:pw, j * TB : j * TB + w], vp[0:pw, 0:w])
            outq.append((v2, bs0, tw, pw))

        def flush_outs():
            for v2d, bs0, otw, opw in outq:
                nc.scalar.dma_start(
                    vout.ap()[bs0 * H2 : bs0 * H2 + opw, 0:otw],
                    v2d[0:opw, :otw],
                )
            outq.clear()

        def seg_pipeline(zts, xh):
            """The final T-segment, fully pipelined: half-0 units run
            while half-1 streams, so only ~2 z-units + the narrow
            stage-2/3 remain after the last input byte lands."""

            def units(us):
                zchains(
                    zts, 0,
                    [
                        (xh[u // 2], (u % 2) * SGU, SGU, TS // 2, u * SGU)
                        for u in us
                    ],
                    pos0=us[0],
                )

            zb = zbp.tile([TB, NTB * PW], fp16, tag="zb")
            v2 = vsb.tile([PW, TPADF], fp16, tag="v2")

            def trans(jlo, jhi):
                for j in range(jlo, jhi):
                    ztp = tpsum.tile([TB, TB], fp16, tag="ztp")
                    nc.tensor.transpose(
                        ztp[:, 0:H2],
                        zts[0:H2, j * TB : (j + 1) * TB],
                        rheye_sb[0:H2, RHF : RHF + H2],
                    )
                    alt_copy(zb[:, j * PW : j * PW + H2], ztp[:, 0:H2])

            def filt(jlo, jhi):
                for j in range(jlo, jhi):
                    vp = vpsum.tile([PW, TB], f32, tag="vp")
                    n_mm = 2 if j > 0 else 1
                    mm = 0
                    for roff, jj in ((0, j - 1), (TB, j)):
                        if jj < 0:
                            continue
                        nc.tensor.matmul(
                            vp[0:H2, :],
                            zb[:, jj * PW : jj * PW + H2],
                            rheye_sb[:, roff : roff + TB],
                            start=(mm == 0),
                            stop=(mm == n_mm - 1),
                        )
                        mm += 1
                    w = min(TB, TS - j * TB)
                    alt_copy(v2[0:H2, j * TB : j * TB + w], vp[0:H2, 0:w])

            units([0, 1])   # half 0: z cols 0..576 -> t-blocks 0..3
            trans(0, 4)
            filt(0, 4)
            units([2, 3])   # half 1 (after its DMA): cols 576..1152
            trans(4, NTBS)
            filt(4, NTBS)
            nc.scalar.dma_start(
                vout.ap()[BPF * H2 : (BPF + 1) * H2, 0:TS], v2[0:H2, 0:TS]
            )

        # The T-segment band is LAST (its narrow 9-block stage-2/3 is
        # the cheapest possible end-of-stream chain); each band's
        # stage-2/3 is emitted after the 2nd b of the NEXT band, and
        # the deferred output DMAs are released once all bulk input
        # DMAs are issued.
        bands = [
            list(range(k * BGRP, (k + 1) * BGRP)) for k in range(BPF // BGRP)
        ] + ["SEG"]
        pending = None
        outq = []
        for gi, bs in enumerate(bands):
            zts = zts_ring[gi % 2]
            if bs == "SEG":
                xh = []
                for h in range(2):
                    xts_t = xspool.tile(
                        [DC, NDC * (TS // 2)], fp8, tag="xts"
                    )
                    nc.gpsimd.dma_start(
                        xts_t[:].rearrange("p (c t) -> p c t", c=NDC),
                        xS.ap()[h],
                    )
                    xh.append(xts_t)
                # all bulk input DMAs are queued: release the first
                # bands' buffered outputs now (they fire as soon as the
                # scalar engine reaches them), the last band's right
                # after its stage-2/3.
                flush_outs()
                if pending is not None:
                    stage23(*pending)
                    pending = None
                flush_outs()
                seg_pipeline(zts, xh)
                continue
            for bq, b in enumerate(bs):
                stage1(b, bq, zts)
                if bq == 1 and pending is not None:
                    stage23(*pending)
                    pending = None
            pending = (bs[0], zts, NTB, T, PW)
        if pending is not None:
            stage23(*pending)

    nc.compile()
    return nc


def _prep_inputs(batch: np.ndarray, W1: np.ndarray, W2: np.ndarray):
    import ml_dtypes

    bf16 = ml_dtypes.bfloat16
    fp16 = np.float16
    fp8 = ml_dtypes.float8_e3m4
    wc = (W2.astype(np.float64) @ W1.astype(np.float64)).astype(np.float64)
    wc = wc / XS  # undo the host pre-scale of x
    # [112, 7*10]: wct[p, c*10+o] = Wc[o, 112c + p]
    wct = np.ascontiguousarray(
        wc.T.reshape(NDC, DC, H2).transpose(1, 0, 2).reshape(DC, NDC * H2)
    ).astype(bf16)
    rheye = np.zeros((TB, CF), np.float32)
    rheye[:, 0:RHF] = _filter_blocks()
    rheye[:, RHF:CF] = np.eye(TB, dtype=np.float32)
    rheye = rheye.astype(fp16)

    xq = (batch * np.float32(XS)).astype(fp8)  # one pass over the f32 data

    # full b's 0..95: [8, 12, 112, 7, 2000]: core, b, d%112 (partitions),
    # d-chunk, t
    xt = np.ascontiguousarray(
        xq[: NCORES * BPF]
        .reshape(NCORES, BPF, T, NDC, DC)
        .transpose(0, 1, 4, 3, 2)
    )

    # T-segments of b's 96..99: core c gets b 96+c//2, half c%2.
    # Lower half: t 0..1152 (host keeps t<1024).  Upper half: t
    # 896..2048 (first 128 are filter warm-up; host keeps t>=1024).
    seg = np.zeros((NCORES, TS, DIN), fp8)
    for c in range(NCORES):
        be = NCORES * BPF + c // 2
        if c % 2 == 0:
            seg[c] = xq[be, 0:TS]
        else:
            seg[c, 0 : T - (SEG_LO - SEG_W0)] = xq[be, SEG_LO - SEG_W0 :]
    xs = np.ascontiguousarray(
        seg.reshape(NCORES, 2, TS // 2, NDC, DC).transpose(0, 1, 4, 3, 2)
    )
    return xt, xs, wct, rheye


def kernel(batch: np.ndarray, W1: np.ndarray, W2: np.ndarray) -> np.ndarray:
    from concourse import bass_utils

    if "nc" not in _CACHE:
        _CACHE["nc"] = _build()
    nc = _CACHE["nc"]

    xt, xs, wct, rheye = _prep_inputs(batch, W1, W2)
    in_maps = [
        {"xT": xt[i], "xS": xs[i], "wct": wct, "rheye": rheye}
        for i in range(NCORES)
    ]
    res = bass_utils.run_bass_kernel_spmd(
        nc, in_maps, core_ids=list(range(NCORES)), **_CACHE.get("run_kwargs", {})
    )
    _CACHE["last_result"] = res

    out = np.empty((B, T, H2), np.float32)
    for c in range(NCORES):
        vo = res.results[c]["vout"].astype(np.float32)  # [130, 2000]
        out[c * BPF : (c + 1) * BPF] = (
            vo[: BPF * H2].reshape(BPF, H2, T).transpose(0, 2, 1)
        )
        segv = vo[BPF * H2 :]  # [10, 2000]; valid cols 0..TS
        be = NCORES * BPF + c // 2
        if c % 2 == 0:
            out[be, 0:SEG_LO] = segv[:, 0:SEG_LO].T
        else:
            out[be, SEG_LO:T] = segv[:, SEG_W0 : SEG_W0 + (T - SEG_LO)].T
    return out
